# revision 1
# baseline (speedup 1.0000x reference)
"""GNN mean-aggregator encoder (GraphSAGE/GCN style) on 8 Trainium2 cores.

Reference computation:
    neigh_mean = mean(features[neigh_idx], axis=1)        # [B, F]
    combined   = concat([features[nodes], neigh_mean], 1) # [B, 2F]
    out        = relu(weight @ combined.T)                # [E, B]

Sharding: data-parallel over the node batch B=50000 across 8 cores (6250
nodes each, padded to 6400 = 50 tiles of 128); features table and weight
replicated per core.

Per 128-node tile on each core:
  1. 17 indirect DMA gathers (one per row slot: self + 16 neighbors) pull
     1KB feature rows into g [128, 17*256] (partition = node).
  2. TensorE identity-matmuls accumulate the 16 neighbor blocks into one
     PSUM region -> neighbor SUM (the 1/16 mean factor is pre-folded into
     the neighbor half of the weight on the host).
  3. ACT copies the sum to SBUF; TensorE transposes self/neigh 128x128
     chunks into PSUM (combined^T), ACT copies to SBUF.
  4. TensorE multiplies with pre-swizzled W^T chunks accumulating over the
     four 128-feature chunks -> psum [128 nodes, 256 emb].
  5. ACT relu-copies to SBUF, DMA to out_t [6400, 256].

Host assembles: concat core outputs' first 6250 rows, transpose -> [256, B].
"""

import numpy as np

P = 128      # nodes per tile / partitions
F = 256      # feature dim
S = 16       # sampled neighbors
E = 256      # embed dim
K = 1 + S    # gathered rows per node
V = 100000   # feature table rows
B_FULL = 50000
N_CORES = 8
B_CORE = B_FULL // N_CORES          # 6250
T = (B_CORE + P - 1) // P           # 50 tiles
B_PAD = T * P                       # 6400

_prog_cache = {}


def _build_program(reps=1):
    import concourse.bass as bass
    import concourse.mybir as mybir
    import concourse.tile as tile
    from concourse import bacc
    from concourse.masks import make_identity

    FP = mybir.dt.float32
    nc = bacc.Bacc("TRN2", num_devices=N_CORES)

    feat = nc.dram_tensor("feat", [V, F], FP, kind="ExternalInput")
    idx_r = nc.dram_tensor("idx_r", [P, T * K], mybir.dt.int32,
                           kind="ExternalInput")
    wt_r = nc.dram_tensor("wt_r", [P, 4 * E], FP, kind="ExternalInput")
    out_t = nc.dram_tensor("out_t", [B_PAD, E], FP, kind="ExternalOutput")

    with tile.TileContext(nc) as tc:
        with tc.tile_pool(name="const", bufs=1) as const, \
             tc.tile_pool(name="gpool", bufs=6) as gpool, \
             tc.tile_pool(name="wpool", bufs=3) as wpool, \
             tc.tile_pool(name="ppool", bufs=2, space="PSUM") as ppool:
            idx_sb = const.tile([P, T * K], mybir.dt.int32)
            nc.sync.dma_start(out=idx_sb[:], in_=idx_r.ap())
            wt_sb = const.tile([P, 4 * E], FP)
            nc.sync.dma_start(out=wt_sb[:], in_=wt_r.ap())
            ident = const.tile([P, P], FP)
            make_identity(nc, ident[:])

            for t in [tt for _ in range(reps) for tt in range(T)]:
                g = gpool.tile([P, K * F], FP, tag="g")
                for j in range(K):
                    nc.gpsimd.indirect_dma_start(
                        out=g[:, j * F:(j + 1) * F], out_offset=None,
                        in_=feat.ap(),
                        in_offset=bass.IndirectOffsetOnAxis(
                            ap=idx_sb[:, t * K + j:t * K + j + 1], axis=0))
                psum_n = ppool.tile([P, F], FP, tag="pn")
                for k in range(S):
                    nc.tensor.matmul(
                        psum_n[:], lhsT=ident[:],
                        rhs=g[:, (1 + k) * F:(2 + k) * F],
                        start=(k == 0), stop=(k == S - 1))
                nsum = wpool.tile([P, F], FP, tag="nsum")
                nc.scalar.activation(nsum[:], psum_n[:],
                                     mybir.ActivationFunctionType.Copy)
                ct_ps = ppool.tile([P, 4 * P], FP, tag="ct")
                for c in range(2):
                    nc.tensor.transpose(ct_ps[:, c * P:(c + 1) * P],
                                        g[:, c * P:(c + 1) * P], ident[:])
                for c in range(2):
                    nc.tensor.transpose(ct_ps[:, (2 + c) * P:(3 + c) * P],
                                        nsum[:, c * P:(c + 1) * P], ident[:])
                ct = wpool.tile([P, 4 * P], FP, tag="ct_sb")
                nc.scalar.activation(ct[:], ct_ps[:],
                                     mybir.ActivationFunctionType.Copy)
                psum_o = ppool.tile([P, E], FP, tag="po")
                for c in range(4):
                    nc.tensor.matmul(
                        psum_o[:], lhsT=ct[:, c * P:(c + 1) * P],
                        rhs=wt_sb[:, c * E:(c + 1) * E],
                        start=(c == 0), stop=(c == 3))
                ot = wpool.tile([P, E], FP, tag="ot")
                nc.scalar.activation(ot[:], psum_o[:],
                                     mybir.ActivationFunctionType.Relu)
                nc.sync.dma_start(out=out_t.ap()[t * P:(t + 1) * P, :],
                                  in_=ot[:])
    nc.compile()
    return nc


def get_program(reps=1):
    key = ("nc", reps)
    if key not in _prog_cache:
        _prog_cache[key] = _build_program(reps)
    return _prog_cache[key]


def _prep_idx(nodes_c, neigh_c):
    """Per-core [b] + [b, S] indices -> tile-swizzled [P, T*K] int32."""
    b = nodes_c.shape[0]
    idx_all = np.zeros((B_PAD, K), np.int32)
    idx_all[:b, 0] = nodes_c
    idx_all[:b, 1:] = neigh_c
    return np.ascontiguousarray(
        idx_all.reshape(T, P, K).transpose(1, 0, 2).reshape(P, T * K))


def _prep_weight(weight):
    """[E, 2F] -> chunk-swizzled W.T [P, 4*E] f32 with mean pre-folded."""
    wt = np.asarray(weight, dtype=np.float32).T.copy()   # [2F, E]
    wt[F:] /= S
    return np.ascontiguousarray(
        wt.reshape(4, P, E).transpose(1, 0, 2).reshape(P, 4 * E))


def make_in_maps(nodes, neigh_idx, features, weight):
    nodes = np.asarray(nodes)
    neigh_idx = np.asarray(neigh_idx)
    features = np.ascontiguousarray(np.asarray(features, dtype=np.float32))
    wt_r = _prep_weight(weight)
    in_maps = []
    for c in range(N_CORES):
        sl = slice(c * B_CORE, (c + 1) * B_CORE)
        idx_r = _prep_idx(nodes[sl].astype(np.int32),
                          neigh_idx[sl].astype(np.int32))
        in_maps.append({"feat": features, "idx_r": idx_r, "wt_r": wt_r})
    return in_maps


def kernel(nodes, neigh_idx, features, weight):
    import concourse.bass_utils as bass_utils

    assert np.asarray(nodes).shape[0] == B_FULL, "kernel hardcodes B=50000"
    nc = get_program()
    in_maps = make_in_maps(nodes, neigh_idx, features, weight)
    res = bass_utils.run_bass_kernel_spmd(
        nc, in_maps, core_ids=list(range(N_CORES)))
    out_t = np.concatenate(
        [res.results[c]["out_t"][:B_CORE] for c in range(N_CORES)], axis=0)
    return np.ascontiguousarray(out_t.T).astype(np.float32)



# revision 4
# speedup vs baseline: 1.0036x; 1.0036x over previous
"""GNN mean-aggregator encoder (GraphSAGE/GCN style) on 8 Trainium2 cores.

Reference computation:
    neigh_mean = mean(features[neigh_idx], axis=1)        # [B, F]
    combined   = concat([features[nodes], neigh_mean], 1) # [B, 2F]
    out        = relu(weight @ combined.T)                # [E, B]

Sharding: data-parallel over the node batch B=50000 across 8 cores (6250
nodes each, padded to 6272 = 49 tiles of 128); features table (bf16) and
weight replicated per core.

The kernel is bound by the SWDGE descriptor-generation fixed cost (~1us
per indirect DMA on the Pool engine): each indirect gather can carry at
most one row index per partition (128 rows), so the 6250*17 = 106250 row
fetches per core need ceil-per-tile 49*17 = 833 gather instructions.
Relative to the 50-tile baseline (850 gathers) this trims the padding
tile and loads the identity from DRAM instead of building it on the Pool
engine.

Per 128-node tile on each core (gathers bf16, PSUM fp32):
  1. 17 indirect-DMA gathers (one per row slot: self + 16 neighbors) pull
     512B feature rows into g [128, 17*256] (partition = node).
  2. TensorE identity-matmuls accumulate the 16 neighbor blocks into one
     PSUM region -> neighbor SUM (the 1/16 mean factor is pre-folded into
     the neighbor half of the weight on the host).
  3. ACT copies the sum to SBUF; TensorE transposes self/neigh 128x128
     chunks into PSUM (combined^T), ACT copies to SBUF as bf16.
  4. TensorE multiplies with pre-swizzled W^T chunks accumulating over the
     four 128-feature chunks -> psum [128 nodes, 256 emb] fp32.
  5. ACT relu-copies to SBUF bf16, DMA to out_t [6272, 256].

Host assembles: concat core outputs' first 6250 rows, transpose, cast to
fp32 -> [256, B].
"""

import numpy as np

P = 128      # nodes per tile / partitions
F = 256      # feature dim
S = 16       # sampled neighbors
E = 256      # embed dim
K = 1 + S    # gathered rows per node
V = 100000   # feature table rows
B_FULL = 50000
N_CORES = 8
B_CORE = B_FULL // N_CORES          # 6250
T = (B_CORE + P - 1) // P           # 49 tiles
B_PAD = T * P                       # 6272

_prog_cache = {}


def _build_program(reps=1):
    import concourse.bass as bass
    import concourse.mybir as mybir
    import concourse.tile as tile
    from concourse import bacc

    FP = mybir.dt.float32
    BF = mybir.dt.bfloat16
    nc = bacc.Bacc("TRN2", num_devices=N_CORES)

    feat = nc.dram_tensor("feat", [V, F], BF, kind="ExternalInput")
    idx_r = nc.dram_tensor("idx_r", [P, T * K], mybir.dt.int32,
                           kind="ExternalInput")
    wt_r = nc.dram_tensor("wt_r", [P, 4 * E], BF, kind="ExternalInput")
    aux_r = nc.dram_tensor("aux_r", [P, P], BF, kind="ExternalInput")
    out_t = nc.dram_tensor("out_t", [B_PAD, E], BF, kind="ExternalOutput")

    with tile.TileContext(nc) as tc:
        with tc.tile_pool(name="const", bufs=1) as const, \
             tc.tile_pool(name="gpool", bufs=6) as gpool, \
             tc.tile_pool(name="wpool", bufs=3) as wpool, \
             tc.tile_pool(name="ppool", bufs=2, space="PSUM") as ppool:
            idx_sb = const.tile([P, T * K], mybir.dt.int32)
            # Head-load the first tile's indices so gathers start early;
            # the rest streams in behind it.
            nc.sync.dma_start(out=idx_sb[:, :K], in_=idx_r.ap()[:, :K])
            nc.sync.dma_start(out=idx_sb[:, K:], in_=idx_r.ap()[:, K:])
            wt_sb = const.tile([P, 4 * E], BF)
            nc.sync.dma_start(out=wt_sb[:], in_=wt_r.ap())
            ident = const.tile([P, P], BF)
            nc.sync.dma_start(out=ident[:], in_=aux_r.ap())

            for t in [tt for _ in range(reps) for tt in range(T)]:
                g = gpool.tile([P, K * F], BF, tag="g")
                ct_ps = ppool.tile([P, 4 * P], FP, tag="ct")
                psum_n = ppool.tile([P, F], FP, tag="pn")
                nc.gpsimd.indirect_dma_start(
                    out=g[:, 0:F], out_offset=None,
                    in_=feat.ap(),
                    in_offset=bass.IndirectOffsetOnAxis(
                        ap=idx_sb[:, t * K:t * K + 1], axis=0))
                for c in range(2):
                    nc.tensor.matmul(
                        ct_ps[:, c * P:(c + 1) * P],
                        lhsT=g[:, c * P:(c + 1) * P], rhs=ident[:],
                        start=True, stop=True)
                # Interleave neighbor gathers with their accumulate matmuls
                # so the PE trails the Pool engine by one slot and the
                # end-of-program tail is short.
                for k in range(S):
                    j = 1 + k
                    nc.gpsimd.indirect_dma_start(
                        out=g[:, j * F:(j + 1) * F], out_offset=None,
                        in_=feat.ap(),
                        in_offset=bass.IndirectOffsetOnAxis(
                            ap=idx_sb[:, t * K + j:t * K + j + 1], axis=0))
                    nc.tensor.matmul(
                        psum_n[:], lhsT=ident[:],
                        rhs=g[:, j * F:(j + 1) * F],
                        start=(k == 0), stop=(k == S - 1))
                nsum = wpool.tile([P, F], BF, tag="nsum")
                nc.scalar.activation(nsum[:], psum_n[:],
                                     mybir.ActivationFunctionType.Copy)
                for c in range(2):
                    nc.tensor.matmul(
                        ct_ps[:, (2 + c) * P:(3 + c) * P],
                        lhsT=nsum[:, c * P:(c + 1) * P], rhs=ident[:],
                        start=True, stop=True)
                ct = wpool.tile([P, 4 * P], BF, tag="ct_sb")
                nc.scalar.activation(ct[:], ct_ps[:],
                                     mybir.ActivationFunctionType.Copy)
                psum_o = ppool.tile([P, E], FP, tag="po")
                for c in range(4):
                    nc.tensor.matmul(
                        psum_o[:], lhsT=ct[:, c * P:(c + 1) * P],
                        rhs=wt_sb[:, c * E:(c + 1) * E],
                        start=(c == 0), stop=(c == 3))
                ot = wpool.tile([P, E], BF, tag="ot")
                nc.scalar.activation(ot[:], psum_o[:],
                                     mybir.ActivationFunctionType.Relu)
                nc.sync.dma_start(out=out_t.ap()[t * P:(t + 1) * P, :],
                                  in_=ot[:])
    nc.compile()
    return nc


def get_program(reps=1):
    key = ("nc", reps)
    if key not in _prog_cache:
        _prog_cache[key] = _build_program(reps)
    return _prog_cache[key]


def _bf16(a):
    import ml_dtypes
    return np.asarray(a, dtype=np.float32).astype(ml_dtypes.bfloat16)


def _prep_idx(nodes_c, neigh_c):
    """Per-core [b] + [b, S] indices -> tile-swizzled [P, T*K] int32."""
    b = nodes_c.shape[0]
    idx_all = np.zeros((B_PAD, K), np.int32)
    idx_all[:b, 0] = nodes_c
    idx_all[:b, 1:] = neigh_c
    return np.ascontiguousarray(
        idx_all.reshape(T, P, K).transpose(1, 0, 2).reshape(P, T * K))


def _prep_weight(weight):
    """[E, 2F] -> chunk-swizzled W.T [P, 4*E] bf16 with mean pre-folded."""
    wt = np.asarray(weight, dtype=np.float32).T.copy()   # [2F, E]
    wt[F:] /= S
    return np.ascontiguousarray(
        _bf16(wt.reshape(4, P, E).transpose(1, 0, 2).reshape(P, 4 * E)))


def make_in_maps(nodes, neigh_idx, features, weight):
    nodes = np.asarray(nodes)
    neigh_idx = np.asarray(neigh_idx)
    features = np.ascontiguousarray(_bf16(features))
    wt_r = _prep_weight(weight)
    aux_r = np.ascontiguousarray(_bf16(np.eye(P, dtype=np.float32)))
    in_maps = []
    for c in range(N_CORES):
        sl = slice(c * B_CORE, (c + 1) * B_CORE)
        idx_r = _prep_idx(nodes[sl].astype(np.int32),
                          neigh_idx[sl].astype(np.int32))
        in_maps.append({"feat": features, "idx_r": idx_r, "wt_r": wt_r,
                        "aux_r": aux_r})
    return in_maps


def kernel(nodes, neigh_idx, features, weight):
    import concourse.bass_utils as bass_utils

    assert np.asarray(nodes).shape[0] == B_FULL, "kernel hardcodes B=50000"
    nc = get_program()
    in_maps = make_in_maps(nodes, neigh_idx, features, weight)
    res = bass_utils.run_bass_kernel_spmd(
        nc, in_maps, core_ids=list(range(N_CORES)))
    out_t = np.concatenate(
        [np.asarray(res.results[c]["out_t"][:B_CORE], dtype=np.float32)
         for c in range(N_CORES)], axis=0)
    return np.ascontiguousarray(out_t.T)


# revision 7
# speedup vs baseline: 1.0040x; 1.0005x over previous
"""GNN mean-aggregator encoder (GraphSAGE/GCN style) on 8 Trainium2 cores.

Reference computation:
    neigh_mean = mean(features[neigh_idx], axis=1)        # [B, F]
    combined   = concat([features[nodes], neigh_mean], 1) # [B, 2F]
    out        = relu(weight @ combined.T)                # [E, B]

Sharding: data-parallel over the node batch B=50000 across 8 cores (6250
nodes each, padded to 6272 = 49 tiles of 128); features table (bf16) and
weight replicated per core.

The kernel is bound by the SWDGE descriptor-generation fixed cost (~1us
per indirect DMA on the Pool engine): each indirect gather can carry at
most one row index per partition (128 rows), so the 6250*17 = 106250 row
fetches per core need ceil-per-tile 49*17 = 833 gather instructions.
Relative to the 50-tile baseline (850 gathers) this trims the padding
tile and loads the identity from DRAM instead of building it on the Pool
engine.

Per 128-node tile on each core (gathers bf16, PSUM fp32):
  1. 17 indirect-DMA gathers (one per row slot: self + 16 neighbors) pull
     512B feature rows into g [128, 17*256] (partition = node).
  2. TensorE identity-matmuls accumulate the 16 neighbor blocks into one
     PSUM region -> neighbor SUM (the 1/16 mean factor is pre-folded into
     the neighbor half of the weight on the host).
  3. ACT copies the sum to SBUF; TensorE transposes self/neigh 128x128
     chunks into PSUM (combined^T), ACT copies to SBUF as bf16.
  4. TensorE multiplies with pre-swizzled W^T chunks accumulating over the
     four 128-feature chunks -> psum [128 nodes, 256 emb] fp32.
  5. ACT relu-copies to SBUF bf16, DMA to out_t [6272, 256].

Host assembles: concat core outputs' first 6250 rows, transpose, cast to
fp32 -> [256, B].
"""

import numpy as np

P = 128      # nodes per tile / partitions
F = 256      # feature dim
S = 16       # sampled neighbors
E = 256      # embed dim
K = 1 + S    # gathered rows per node
V = 100000   # feature table rows
B_FULL = 50000
N_CORES = 8
B_CORE = B_FULL // N_CORES          # 6250
T = (B_CORE + P - 1) // P           # 49 tiles
B_PAD = T * P                       # 6272

_prog_cache = {}


def _build_program(reps=1):
    import concourse.bass as bass
    import concourse.mybir as mybir
    import concourse.tile as tile
    from concourse import bacc

    FP = mybir.dt.float32
    BF = mybir.dt.bfloat16
    nc = bacc.Bacc("TRN2", num_devices=N_CORES)

    feat = nc.dram_tensor("feat", [V, F], BF, kind="ExternalInput")
    idx_r = nc.dram_tensor("idx_r", [P, T * K], mybir.dt.int32,
                           kind="ExternalInput")
    wt_r = nc.dram_tensor("wt_r", [P, 4 * E], BF, kind="ExternalInput")
    aux_r = nc.dram_tensor("aux_r", [P, P], BF, kind="ExternalInput")
    out_t = nc.dram_tensor("out_t", [B_PAD, E], BF, kind="ExternalOutput")

    with tile.TileContext(nc) as tc:
        with tc.tile_pool(name="const", bufs=1) as const, \
             tc.tile_pool(name="gpool", bufs=6) as gpool, \
             tc.tile_pool(name="wpool", bufs=3) as wpool, \
             tc.tile_pool(name="ppool", bufs=2, space="PSUM") as ppool:
            idx_sb = const.tile([P, T * K], mybir.dt.int32)
            # Head-load the first tile's indices so gathers start early;
            # the rest streams in behind it.
            nc.sync.dma_start(out=idx_sb[:, :K], in_=idx_r.ap()[:, :K])
            nc.sync.dma_start(out=idx_sb[:, K:], in_=idx_r.ap()[:, K:])
            wt_sb = const.tile([P, 4 * E], BF)
            nc.sync.dma_start(out=wt_sb[:], in_=wt_r.ap())
            ident = const.tile([P, P], BF)
            nc.sync.dma_start(out=ident[:], in_=aux_r.ap())

            for t in [tt for _ in range(reps) for tt in range(T)]:
                g = gpool.tile([P, K * F], BF, tag="g")
                ct_ps = ppool.tile([P, 4 * P], FP, tag="ct")
                nc.gpsimd.indirect_dma_start(
                    out=g[:, 0:F], out_offset=None,
                    in_=feat.ap(),
                    in_offset=bass.IndirectOffsetOnAxis(
                        ap=idx_sb[:, t * K:t * K + 1], axis=0))
                # One PSUM accumulation group covers the whole ct_ps bank:
                # the first matmul opens it (start), the last closes it
                # (stop); in between, the first touch of each byte range
                # after the start writes (lazy zero) and repeat touches
                # accumulate — giving self transposes (touched once) and
                # neighbor transpose-accumulates (touched 16x) in one pass.
                for c in range(2):
                    nc.tensor.matmul(
                        ct_ps[:, c * P:(c + 1) * P],
                        lhsT=g[:, c * P:(c + 1) * P], rhs=ident[:],
                        start=(c == 0), stop=False, skip_group_check=True)
                ct = wpool.tile([P, 4 * P], BF, tag="ct_sb")
                # Self half of combined^T is complete already — copy it out
                # and start the finals' self-chunk matmuls while neighbor
                # gathers still stream; only the neighbor half of the copy
                # and the last two final chunks remain on the tail.
                nc.scalar.activation(ct[:, 0:2 * P], ct_ps[:, 0:2 * P],
                                     mybir.ActivationFunctionType.Copy)
                psum_o = ppool.tile([P, E], FP, tag="po")
                ot = wpool.tile([P, E], BF, tag="ot")
                # Interleave neighbor gathers with transpose-accumulate
                # matmuls so the PE trails the Pool engine by one gather and
                # the end-of-program tail is short.
                for k in range(S):
                    j = 1 + k
                    nc.gpsimd.indirect_dma_start(
                        out=g[:, j * F:(j + 1) * F], out_offset=None,
                        in_=feat.ap(),
                        in_offset=bass.IndirectOffsetOnAxis(
                            ap=idx_sb[:, t * K + j:t * K + j + 1], axis=0))
                    for c in range(2):
                        nc.tensor.matmul(
                            ct_ps[:, (2 + c) * P:(3 + c) * P],
                            lhsT=g[:, j * F + c * P:j * F + (c + 1) * P],
                            rhs=ident[:],
                            start=False, stop=(k == S - 1 and c == 1),
                            skip_group_check=True)
                    if k < 2:
                        # Final-matmul self chunk k for both embed halves
                        # (single psum_o accumulation group: one start, one
                        # stop, lazy-zero handles each half's first touch).
                        for h in range(2):
                            nc.tensor.matmul(
                                psum_o[:, h * P:(h + 1) * P],
                                lhsT=ct[:, k * P:(k + 1) * P],
                                rhs=wt_sb[:, k * E + h * P:k * E + (h + 1) * P],
                                start=(k == 0 and h == 0), stop=False,
                                skip_group_check=True)
                nc.scalar.activation(ct[:, 2 * P:4 * P], ct_ps[:, 2 * P:4 * P],
                                     mybir.ActivationFunctionType.Copy)
                # Remaining final chunks (neighbor half), then per-half
                # relu/store so the first store overlaps the second half.
                for h in range(2):
                    for c in range(2, 4):
                        nc.tensor.matmul(
                            psum_o[:, h * P:(h + 1) * P],
                            lhsT=ct[:, c * P:(c + 1) * P],
                            rhs=wt_sb[:, c * E + h * P:c * E + (h + 1) * P],
                            start=False, stop=(h == 1 and c == 3),
                            skip_group_check=True)
                    nc.scalar.activation(ot[:, h * P:(h + 1) * P],
                                         psum_o[:, h * P:(h + 1) * P],
                                         mybir.ActivationFunctionType.Relu)
                    nc.sync.dma_start(
                        out=out_t.ap()[t * P:(t + 1) * P, h * P:(h + 1) * P],
                        in_=ot[:, h * P:(h + 1) * P])
    nc.compile()
    return nc


def get_program(reps=1):
    key = ("nc", reps)
    if key not in _prog_cache:
        _prog_cache[key] = _build_program(reps)
    return _prog_cache[key]


def _bf16(a):
    import ml_dtypes
    return np.asarray(a, dtype=np.float32).astype(ml_dtypes.bfloat16)


def _prep_idx(nodes_c, neigh_c):
    """Per-core [b] + [b, S] indices -> tile-swizzled [P, T*K] int32."""
    b = nodes_c.shape[0]
    idx_all = np.zeros((B_PAD, K), np.int32)
    idx_all[:b, 0] = nodes_c
    idx_all[:b, 1:] = neigh_c
    return np.ascontiguousarray(
        idx_all.reshape(T, P, K).transpose(1, 0, 2).reshape(P, T * K))


def _prep_weight(weight):
    """[E, 2F] -> chunk-swizzled W.T [P, 4*E] bf16 with mean pre-folded."""
    wt = np.asarray(weight, dtype=np.float32).T.copy()   # [2F, E]
    wt[F:] /= S
    return np.ascontiguousarray(
        _bf16(wt.reshape(4, P, E).transpose(1, 0, 2).reshape(P, 4 * E)))


def make_in_maps(nodes, neigh_idx, features, weight):
    nodes = np.asarray(nodes)
    neigh_idx = np.asarray(neigh_idx)
    features = np.ascontiguousarray(_bf16(features))
    wt_r = _prep_weight(weight)
    aux_r = np.ascontiguousarray(_bf16(np.eye(P, dtype=np.float32)))
    in_maps = []
    for c in range(N_CORES):
        sl = slice(c * B_CORE, (c + 1) * B_CORE)
        idx_r = _prep_idx(nodes[sl].astype(np.int32),
                          neigh_idx[sl].astype(np.int32))
        in_maps.append({"feat": features, "idx_r": idx_r, "wt_r": wt_r,
                        "aux_r": aux_r})
    return in_maps


def kernel(nodes, neigh_idx, features, weight):
    import concourse.bass_utils as bass_utils

    assert np.asarray(nodes).shape[0] == B_FULL, "kernel hardcodes B=50000"
    nc = get_program()
    in_maps = make_in_maps(nodes, neigh_idx, features, weight)
    res = bass_utils.run_bass_kernel_spmd(
        nc, in_maps, core_ids=list(range(N_CORES)))
    out_t = np.concatenate(
        [np.asarray(res.results[c]["out_t"][:B_CORE], dtype=np.float32)
         for c in range(N_CORES)], axis=0)
    return np.ascontiguousarray(out_t.T)


# revision 8
# speedup vs baseline: 1.0045x; 1.0005x over previous
"""GNN mean-aggregator encoder (GraphSAGE/GCN style) on 8 Trainium2 cores.

Reference computation:
    neigh_mean = mean(features[neigh_idx], axis=1)        # [B, F]
    combined   = concat([features[nodes], neigh_mean], 1) # [B, 2F]
    out        = relu(weight @ combined.T)                # [E, B]

Sharding: data-parallel over the node batch B=50000 across 8 cores (6250
nodes each, padded to 6272 = 49 tiles of 128); features table (bf16) and
weight replicated per core.

The kernel is bound by the SWDGE descriptor-generation fixed cost (~1us
per indirect DMA on the Pool engine): each indirect gather can carry at
most one row index per partition (128 rows), so the 6250*17 = 106250 row
fetches per core need ceil-per-tile 49*17 = 833 gather instructions.
Relative to the 50-tile baseline (850 gathers) this trims the padding
tile and loads the identity from DRAM instead of building it on the Pool
engine.

Per 128-node tile on each core (gathers bf16, PSUM fp32):
  1. 17 indirect-DMA gathers (one per row slot: self + 16 neighbors) pull
     512B feature rows into g [128, 17*256] (partition = node); each
     neighbor gather is chased by its transpose-accumulate matmuls so the
     PE trails the Pool engine by one slot.
  2. TensorE matmuls with an identity rhs transpose the self chunks and
     transpose-ACCUMULATE the neighbor chunks into one PSUM bank
     (combined^T; one accumulation group: single start/stop, lazy zero
     covers each region's first touch; the 1/16 mean factor is pre-folded
     into the neighbor half of the weight on the host).
  3. ACT copies the self half of combined^T to SBUF early (its finals run
     under the neighbor gathers); the neighbor half follows at tile end.
  4. TensorE multiplies with pre-swizzled W^T chunks accumulating over the
     four 128-feature chunks -> psum [128 nodes, 256 emb] fp32, split in
     two embed halves so relu/store of half 0 overlaps half 1.
  5. ACT relu-copies to SBUF bf16, DMA to out_t [6272, 256].

Host assembles: concat core outputs' first 6250 rows, transpose, cast to
fp32 -> [256, B].
"""

import numpy as np

P = 128      # nodes per tile / partitions
F = 256      # feature dim
S = 16       # sampled neighbors
E = 256      # embed dim
K = 1 + S    # gathered rows per node
V = 100000   # feature table rows
B_FULL = 50000
N_CORES = 8
B_CORE = B_FULL // N_CORES          # 6250
T = (B_CORE + P - 1) // P           # 49 tiles
B_PAD = T * P                       # 6272

_prog_cache = {}


def _build_program(reps=1):
    import concourse.bass as bass
    import concourse.mybir as mybir
    import concourse.tile as tile
    from concourse import bacc

    FP = mybir.dt.float32
    BF = mybir.dt.bfloat16
    nc = bacc.Bacc("TRN2", num_devices=N_CORES)

    feat = nc.dram_tensor("feat", [V, F], BF, kind="ExternalInput")
    idx_r = nc.dram_tensor("idx_r", [P, T * K], mybir.dt.int32,
                           kind="ExternalInput")
    wt_r = nc.dram_tensor("wt_r", [P, 4 * E], BF, kind="ExternalInput")
    aux_r = nc.dram_tensor("aux_r", [P, P], BF, kind="ExternalInput")
    out_t = nc.dram_tensor("out_t", [B_PAD, E], BF, kind="ExternalOutput")

    with tile.TileContext(nc) as tc:
        with tc.tile_pool(name="const", bufs=1) as const, \
             tc.tile_pool(name="gpool", bufs=6) as gpool, \
             tc.tile_pool(name="wpool", bufs=3) as wpool, \
             tc.tile_pool(name="ppool", bufs=2, space="PSUM") as ppool:
            idx_sb = const.tile([P, T * K], mybir.dt.int32)
            # Head-load the first tile's indices so gathers start early;
            # the rest streams in behind it.
            nc.sync.dma_start(out=idx_sb[:, :K], in_=idx_r.ap()[:, :K])
            nc.sync.dma_start(out=idx_sb[:, K:], in_=idx_r.ap()[:, K:])
            wt_sb = const.tile([P, 4 * E], BF)
            nc.sync.dma_start(out=wt_sb[:], in_=wt_r.ap())
            ident = const.tile([P, P], BF)
            nc.sync.dma_start(out=ident[:], in_=aux_r.ap())

            for t in [tt for _ in range(reps) for tt in range(T)]:
                g = gpool.tile([P, K * F], BF, tag="g")
                ct_ps = ppool.tile([P, 4 * P], FP, tag="ct")
                nc.gpsimd.indirect_dma_start(
                    out=g[:, 0:F], out_offset=None,
                    in_=feat.ap(),
                    in_offset=bass.IndirectOffsetOnAxis(
                        ap=idx_sb[:, t * K:t * K + 1], axis=0))
                # One PSUM accumulation group covers the whole ct_ps bank:
                # the first matmul opens it (start), the last closes it
                # (stop); in between, the first touch of each byte range
                # after the start writes (lazy zero) and repeat touches
                # accumulate — giving self transposes (touched once) and
                # neighbor transpose-accumulates (touched 16x) in one pass.
                for c in range(2):
                    nc.tensor.matmul(
                        ct_ps[:, c * P:(c + 1) * P],
                        lhsT=g[:, c * P:(c + 1) * P], rhs=ident[:],
                        start=(c == 0), stop=False, skip_group_check=True)
                ct = wpool.tile([P, 4 * P], BF, tag="ct_sb")
                # Self half of combined^T is complete already — copy it out
                # and start the finals' self-chunk matmuls while neighbor
                # gathers still stream; only the neighbor half of the copy
                # and the last two final chunks remain on the tail.
                nc.scalar.activation(ct[:, 0:2 * P], ct_ps[:, 0:2 * P],
                                     mybir.ActivationFunctionType.Copy)
                psum_o = ppool.tile([P, E], FP, tag="po")
                ot = wpool.tile([P, E], BF, tag="ot")
                # Interleave neighbor gathers with transpose-accumulate
                # matmuls so the PE trails the Pool engine by one gather and
                # the end-of-program tail is short.
                for k in range(S):
                    j = 1 + k
                    nc.gpsimd.indirect_dma_start(
                        out=g[:, j * F:(j + 1) * F], out_offset=None,
                        in_=feat.ap(),
                        in_offset=bass.IndirectOffsetOnAxis(
                            ap=idx_sb[:, t * K + j:t * K + j + 1], axis=0))
                    for c in range(2):
                        nc.tensor.matmul(
                            ct_ps[:, (2 + c) * P:(3 + c) * P],
                            lhsT=g[:, j * F + c * P:j * F + (c + 1) * P],
                            rhs=ident[:],
                            start=False, stop=(k == S - 1 and c == 1),
                            skip_group_check=True)
                    if k < 2:
                        # Final-matmul self chunk k for both embed halves
                        # (single psum_o accumulation group: one start, one
                        # stop, lazy-zero handles each half's first touch).
                        for h in range(2):
                            nc.tensor.matmul(
                                psum_o[:, h * P:(h + 1) * P],
                                lhsT=ct[:, k * P:(k + 1) * P],
                                rhs=wt_sb[:, k * E + h * P:k * E + (h + 1) * P],
                                start=(k == 0 and h == 0), stop=False,
                                skip_group_check=True)
                nc.scalar.activation(ct[:, 2 * P:4 * P], ct_ps[:, 2 * P:4 * P],
                                     mybir.ActivationFunctionType.Copy)
                # Remaining final chunks (neighbor half), then per-half
                # relu/store so the first store overlaps the second half.
                for h in range(2):
                    for c in range(2, 4):
                        nc.tensor.matmul(
                            psum_o[:, h * P:(h + 1) * P],
                            lhsT=ct[:, c * P:(c + 1) * P],
                            rhs=wt_sb[:, c * E + h * P:c * E + (h + 1) * P],
                            start=False, stop=(h == 1 and c == 3),
                            skip_group_check=True)
                    nc.scalar.activation(ot[:, h * P:(h + 1) * P],
                                         psum_o[:, h * P:(h + 1) * P],
                                         mybir.ActivationFunctionType.Relu)
                    nc.sync.dma_start(
                        out=out_t.ap()[t * P:(t + 1) * P, h * P:(h + 1) * P],
                        in_=ot[:, h * P:(h + 1) * P])
    nc.compile()
    return nc


def get_program(reps=1):
    key = ("nc", reps)
    if key not in _prog_cache:
        _prog_cache[key] = _build_program(reps)
    return _prog_cache[key]


def _bf16(a):
    import ml_dtypes
    return np.asarray(a, dtype=np.float32).astype(ml_dtypes.bfloat16)


def _prep_idx(nodes_c, neigh_c):
    """Per-core [b] + [b, S] indices -> tile-swizzled [P, T*K] int32."""
    b = nodes_c.shape[0]
    idx_all = np.zeros((B_PAD, K), np.int32)
    idx_all[:b, 0] = nodes_c
    idx_all[:b, 1:] = neigh_c
    return np.ascontiguousarray(
        idx_all.reshape(T, P, K).transpose(1, 0, 2).reshape(P, T * K))


def _prep_weight(weight):
    """[E, 2F] -> chunk-swizzled W.T [P, 4*E] bf16 with mean pre-folded."""
    wt = np.asarray(weight, dtype=np.float32).T.copy()   # [2F, E]
    wt[F:] /= S
    return np.ascontiguousarray(
        _bf16(wt.reshape(4, P, E).transpose(1, 0, 2).reshape(P, 4 * E)))


def make_in_maps(nodes, neigh_idx, features, weight):
    nodes = np.asarray(nodes)
    neigh_idx = np.asarray(neigh_idx)
    features = np.ascontiguousarray(_bf16(features))
    wt_r = _prep_weight(weight)
    aux_r = np.ascontiguousarray(_bf16(np.eye(P, dtype=np.float32)))
    in_maps = []
    for c in range(N_CORES):
        sl = slice(c * B_CORE, (c + 1) * B_CORE)
        idx_r = _prep_idx(nodes[sl].astype(np.int32),
                          neigh_idx[sl].astype(np.int32))
        in_maps.append({"feat": features, "idx_r": idx_r, "wt_r": wt_r,
                        "aux_r": aux_r})
    return in_maps


def kernel(nodes, neigh_idx, features, weight):
    import concourse.bass_utils as bass_utils

    assert np.asarray(nodes).shape[0] == B_FULL, "kernel hardcodes B=50000"
    nc = get_program()
    in_maps = make_in_maps(nodes, neigh_idx, features, weight)
    res = bass_utils.run_bass_kernel_spmd(
        nc, in_maps, core_ids=list(range(N_CORES)))
    out_t = np.concatenate(
        [np.asarray(res.results[c]["out_t"][:B_CORE], dtype=np.float32)
         for c in range(N_CORES)], axis=0)
    return np.ascontiguousarray(out_t.T)


# revision 19
# speedup vs baseline: 1.0066x; 1.0021x over previous
"""GNN mean-aggregator encoder (GraphSAGE/GCN style) on 8 Trainium2 cores.

Reference computation:
    neigh_mean = mean(features[neigh_idx], axis=1)        # [B, F]
    combined   = concat([features[nodes], neigh_mean], 1) # [B, 2F]
    out        = relu(weight @ combined.T)                # [E, B]

Sharding: data-parallel over the node batch B=50000 across 8 cores (6250
nodes each, padded to 6272 = 49 tiles of 128); features table (bf16) and
weight replicated per core.

The kernel is bound by the SWDGE descriptor-generation fixed cost (~1us
per indirect DMA on the Pool engine): each indirect gather can carry at
most one row index per partition (128 rows), so the 6250*17 = 106250 row
fetches per core need ceil-per-tile 49*17 = 833 gather instructions.
Relative to the 50-tile baseline (850 gathers) this trims the padding
tile and loads the identity from DRAM instead of building it on the Pool
engine.

Per 128-node tile on each core (gathers bf16, PSUM fp32):
  1. 17 indirect-DMA gathers (one per row slot: self + 16 neighbors) pull
     512B feature rows into g [128, 17*256] (partition = node); each
     neighbor gather is chased by its transpose-accumulate matmuls so the
     PE trails the Pool engine by one slot.
  2. TensorE matmuls with an identity rhs transpose the self chunks and
     transpose-ACCUMULATE the neighbor chunks into one PSUM bank
     (combined^T; one accumulation group: single start/stop, lazy zero
     covers each region's first touch; the 1/16 mean factor is pre-folded
     into the neighbor half of the weight on the host).
  3. ACT copies the self half of combined^T to SBUF early (its finals run
     under the neighbor gathers); the neighbor half follows at tile end.
  4. TensorE multiplies with pre-swizzled W^T chunks accumulating over the
     four 128-feature chunks -> psum [128 nodes, 256 emb] fp32, split in
     two embed halves so relu/store of half 0 overlaps half 1.
  5. ACT relu-copies to SBUF bf16, DMA to out_t [6272, 256].

Host assembles: concat core outputs' first 6250 rows, transpose, cast to
fp32 -> [256, B].
"""

import numpy as np

P = 128      # nodes per tile / partitions
F = 256      # feature dim
S = 16       # sampled neighbors
E = 256      # embed dim
K = 1 + S    # gathered rows per node
V = 100000   # feature table rows
B_FULL = 50000
N_CORES = 8
B_CORE = B_FULL // N_CORES          # 6250
T = (B_CORE + P - 1) // P           # 49 tiles
B_PAD = T * P                       # 6272
TF = B_CORE // P                    # 48 full tiles
R_NODES = B_CORE - TF * P           # 106 nodes in the runt tile
R_TOK = R_NODES * K                 # 1802 runt tokens
NB = (R_TOK + P - 1) // P           # 15 node-major runt gather blocks
NCOL = TF * K + NB                  # 831 index columns

_prog_cache = {}


def _build_program(reps=1):
    import concourse.bass as bass
    import concourse.mybir as mybir
    import concourse.tile as tile
    from concourse import bacc

    FP = mybir.dt.float32
    BF = mybir.dt.bfloat16
    nc = bacc.Bacc("TRN2", num_devices=N_CORES)

    feat = nc.dram_tensor("feat", [V, F], BF, kind="ExternalInput")
    idx_r = nc.dram_tensor("idx_r", [P, NCOL], mybir.dt.int32,
                           kind="ExternalInput")
    wt_r = nc.dram_tensor("wt_r", [P, 4 * E], BF, kind="ExternalInput")
    # aux: identity [P, P] ++ per-runt-block selection matrices
    # [P, NB * 18] (9 self-router + 9 neighbor-sum columns per block)
    # ++ zeros [P, P] (lhsT of the runt PSUM-bank pre-zeroing matmul).
    aux_r = nc.dram_tensor("aux_r", [P, P + NB * 18 + P], BF,
                           kind="ExternalInput")
    out_t = nc.dram_tensor("out_t", [B_PAD, E], BF, kind="ExternalOutput")

    with tile.TileContext(nc) as tc:
        with tc.tile_pool(name="const", bufs=1) as const, \
             tc.tile_pool(name="gpool", bufs=6) as gpool, \
             tc.tile_pool(name="wpool", bufs=3) as wpool, \
             tc.tile_pool(name="ppool", bufs=2, space="PSUM") as ppool:
            idx_sb = const.tile([P, NCOL], mybir.dt.int32)
            # Head-load the first tile's indices so gathers start early;
            # the rest streams in behind it.
            nc.sync.dma_start(out=idx_sb[:, :K], in_=idx_r.ap()[:, :K])
            nc.sync.dma_start(out=idx_sb[:, K:], in_=idx_r.ap()[:, K:])
            wt_sb = const.tile([P, 4 * E], BF)
            nc.sync.dma_start(out=wt_sb[:], in_=wt_r.ap())
            ident = const.tile([P, P], BF)
            nc.sync.dma_start(out=ident[:], in_=aux_r.ap()[:, 0:P])
            rsel = const.tile([P, NB * 18], BF)
            nc.sync.dma_start(out=rsel[:], in_=aux_r.ap()[:, P:P + NB * 18])
            zeros = const.tile([P, P], BF)
            nc.sync.dma_start(out=zeros[:], in_=aux_r.ap()[:, P + NB * 18:])

            for rep in range(reps):
              for t in range(TF):
                g = gpool.tile([P, K * F], BF, tag="g")
                ct_ps = ppool.tile([P, 4 * P], FP, tag="ct")
                nc.gpsimd.indirect_dma_start(
                    out=g[:, 0:F], out_offset=None,
                    in_=feat.ap(),
                    in_offset=bass.IndirectOffsetOnAxis(
                        ap=idx_sb[:, t * K:t * K + 1], axis=0))
                # One PSUM accumulation group covers the whole ct_ps bank:
                # the first matmul opens it (start), the last closes it
                # (stop); in between, the first touch of each byte range
                # after the start writes (lazy zero) and repeat touches
                # accumulate — giving self transposes (touched once) and
                # neighbor transpose-accumulates (touched 16x) in one pass.
                for c in range(2):
                    nc.tensor.matmul(
                        ct_ps[:, c * P:(c + 1) * P],
                        lhsT=g[:, c * P:(c + 1) * P], rhs=ident[:],
                        start=(c == 0), stop=False, skip_group_check=True)
                ct = wpool.tile([P, 4 * P], BF, tag="ct_sb")
                # Self half of combined^T is complete already — copy it out
                # and start the finals' self-chunk matmuls while neighbor
                # gathers still stream; only the neighbor half of the copy
                # and the last two final chunks remain on the tail.
                nc.scalar.activation(ct[:, 0:2 * P], ct_ps[:, 0:2 * P],
                                     mybir.ActivationFunctionType.Copy)
                psum_o = ppool.tile([P, E], FP, tag="po")
                ot = wpool.tile([P, E], BF, tag="ot")
                # Interleave neighbor gathers with transpose-accumulate
                # matmuls so the PE trails the Pool engine by one gather and
                # the end-of-program tail is short.
                for k in range(S):
                    j = 1 + k
                    nc.gpsimd.indirect_dma_start(
                        out=g[:, j * F:(j + 1) * F], out_offset=None,
                        in_=feat.ap(),
                        in_offset=bass.IndirectOffsetOnAxis(
                            ap=idx_sb[:, t * K + j:t * K + j + 1], axis=0))
                    for c in range(2):
                        nc.tensor.matmul(
                            ct_ps[:, (2 + c) * P:(3 + c) * P],
                            lhsT=g[:, j * F + c * P:j * F + (c + 1) * P],
                            rhs=ident[:],
                            start=False, stop=(k == S - 1 and c == 1),
                            skip_group_check=True)
                    if k < 2:
                        # Final-matmul self chunk k for both embed halves
                        # (single psum_o accumulation group: one start, one
                        # stop, lazy-zero handles each half's first touch).
                        for h in range(2):
                            nc.tensor.matmul(
                                psum_o[:, h * P:(h + 1) * P],
                                lhsT=ct[:, k * P:(k + 1) * P],
                                rhs=wt_sb[:, k * E + h * P:k * E + (h + 1) * P],
                                start=(k == 0 and h == 0), stop=False,
                                skip_group_check=True)
                nc.scalar.activation(ct[:, 2 * P:4 * P], ct_ps[:, 2 * P:4 * P],
                                     mybir.ActivationFunctionType.Copy)
                # Remaining final chunks (neighbor half), then per-half
                # relu/store so the first store overlaps the second half.
                for h in range(2):
                    for c in range(2, 4):
                        nc.tensor.matmul(
                            psum_o[:, h * P:(h + 1) * P],
                            lhsT=ct[:, c * P:(c + 1) * P],
                            rhs=wt_sb[:, c * E + h * P:c * E + (h + 1) * P],
                            start=False, stop=(h == 1 and c == 3),
                            skip_group_check=True)
                    nc.scalar.activation(ot[:, h * P:(h + 1) * P],
                                         psum_o[:, h * P:(h + 1) * P],
                                         mybir.ActivationFunctionType.Relu)
                    nc.sync.dma_start(
                        out=out_t.ap()[t * P:(t + 1) * P, h * P:(h + 1) * P],
                        in_=ot[:, h * P:(h + 1) * P])

              # Runt tile (106 nodes, 1802 tokens): tokens are node-major
              # packed so only ceil(1802/128)=15 gathers are needed instead
              # of 17. Constant 0/1 selection matrices (rsel) route each
              # token's transposed features to its node column: self tokens
              # via a one-hot router, neighbor tokens summed by column. The
              # bank is pre-zeroed by one zeros-lhsT matmul so every
              # selection matmul is a pure accumulate (a node's 17 tokens
              # can straddle two gather blocks).
              g = gpool.tile([P, K * F], BF, tag="g")
              ct_ps = ppool.tile([P, 4 * P], FP, tag="ct")
              nc.tensor.matmul(
                  ct_ps[:], lhsT=zeros[:], rhs=wt_sb[:, 0:4 * P],
                  start=True, stop=False, skip_group_check=True)
              for q in range(NB):
                  nc.gpsimd.indirect_dma_start(
                      out=g[:, q * F:(q + 1) * F], out_offset=None,
                      in_=feat.ap(),
                      in_offset=bass.IndirectOffsetOnAxis(
                          ap=idx_sb[:, TF * K + q:TF * K + q + 1], axis=0))
                  base = (P * q) // K
                  w = min(P * q + P - 1, R_TOK - 1) // K - base + 1
                  for c in range(2):
                      lhs = g[:, q * F + c * P:q * F + (c + 1) * P]
                      nc.tensor.matmul(
                          ct_ps[:, c * P + base:c * P + base + w],
                          lhsT=lhs, rhs=rsel[:, q * 18:q * 18 + w],
                          start=False, stop=False, skip_group_check=True)
                      nc.tensor.matmul(
                          ct_ps[:, (2 + c) * P + base:(2 + c) * P + base + w],
                          lhsT=lhs, rhs=rsel[:, q * 18 + 9:q * 18 + 9 + w],
                          start=False,
                          stop=(q == NB - 1 and c == 1),
                          skip_group_check=True)
              ct = wpool.tile([P, 4 * P], BF, tag="ct_sb")
              nc.scalar.activation(ct[:], ct_ps[:],
                                   mybir.ActivationFunctionType.Copy)
              psum_o = ppool.tile([P, E], FP, tag="po")
              ot = wpool.tile([P, E], BF, tag="ot")
              for h in range(2):
                  for c in range(4):
                      nc.tensor.matmul(
                          psum_o[:, h * P:(h + 1) * P],
                          lhsT=ct[:, c * P:(c + 1) * P],
                          rhs=wt_sb[:, c * E + h * P:c * E + (h + 1) * P],
                          start=(h == 0 and c == 0),
                          stop=(h == 1 and c == 3),
                          skip_group_check=True)
                  nc.scalar.activation(ot[:, h * P:(h + 1) * P],
                                       psum_o[:, h * P:(h + 1) * P],
                                       mybir.ActivationFunctionType.Relu)
                  nc.sync.dma_start(
                      out=out_t.ap()[TF * P:T * P, h * P:(h + 1) * P],
                      in_=ot[:, h * P:(h + 1) * P])
    nc.compile()
    return nc


def get_program(reps=1):
    key = ("nc", reps)
    if key not in _prog_cache:
        _prog_cache[key] = _build_program(reps)
    return _prog_cache[key]


def _bf16(a):
    import ml_dtypes
    return np.asarray(a, dtype=np.float32).astype(ml_dtypes.bfloat16)


def _prep_idx(nodes_c, neigh_c):
    """Per-core [b] + [b, S] indices -> [P, NCOL] int32: 48 slot-major
    full tiles followed by NB node-major-packed runt gather columns."""
    idx_all = np.concatenate([nodes_c[:, None], neigh_c], axis=1)  # [b, K]
    full = (idx_all[:TF * P]
            .reshape(TF, P, K).transpose(1, 0, 2).reshape(P, TF * K))
    # Runt: token i (node-major) -> partition i % P, column i // P.
    toks = np.zeros(NB * P, np.int32)
    toks[:R_TOK] = idx_all[TF * P:].reshape(-1)
    runt = toks.reshape(NB, P).T
    return np.ascontiguousarray(
        np.concatenate([full, runt], axis=1).astype(np.int32))


def _prep_aux():
    """identity ++ runt selection matrices ++ zeros, [P, P+NB*18+P] bf16."""
    rsel = np.zeros((P, NB * 18), np.float32)
    for q in range(NB):
        base = (P * q) // K
        for p in range(P):
            i = P * q + p
            if i >= R_TOK:
                continue
            n, s = divmod(i, K)
            rsel[p, q * 18 + (0 if s == 0 else 9) + (n - base)] = 1.0
    return np.ascontiguousarray(_bf16(np.concatenate(
        [np.eye(P, dtype=np.float32), rsel, np.zeros((P, P), np.float32)],
        axis=1)))


def _prep_weight(weight):
    """[E, 2F] -> chunk-swizzled W.T [P, 4*E] bf16 with mean pre-folded."""
    wt = np.asarray(weight, dtype=np.float32).T.copy()   # [2F, E]
    wt[F:] /= S
    return np.ascontiguousarray(
        _bf16(wt.reshape(4, P, E).transpose(1, 0, 2).reshape(P, 4 * E)))


def make_in_maps(nodes, neigh_idx, features, weight):
    nodes = np.asarray(nodes)
    neigh_idx = np.asarray(neigh_idx)
    features = np.ascontiguousarray(_bf16(features))
    wt_r = _prep_weight(weight)
    aux_r = _prep_aux()
    in_maps = []
    for c in range(N_CORES):
        sl = slice(c * B_CORE, (c + 1) * B_CORE)
        idx_r = _prep_idx(nodes[sl].astype(np.int32),
                          neigh_idx[sl].astype(np.int32))
        in_maps.append({"feat": features, "idx_r": idx_r, "wt_r": wt_r,
                        "aux_r": aux_r})
    return in_maps


def kernel(nodes, neigh_idx, features, weight):
    import concourse.bass_utils as bass_utils

    assert np.asarray(nodes).shape[0] == B_FULL, "kernel hardcodes B=50000"
    nc = get_program()
    in_maps = make_in_maps(nodes, neigh_idx, features, weight)
    res = bass_utils.run_bass_kernel_spmd(
        nc, in_maps, core_ids=list(range(N_CORES)))
    out_t = np.concatenate(
        [np.asarray(res.results[c]["out_t"][:B_CORE], dtype=np.float32)
         for c in range(N_CORES)], axis=0)
    return np.ascontiguousarray(out_t.T)


# revision 22
# speedup vs baseline: 1.0070x; 1.0004x over previous
"""GNN mean-aggregator encoder (GraphSAGE/GCN style) on 8 Trainium2 cores.

Reference computation:
    neigh_mean = mean(features[neigh_idx], axis=1)        # [B, F]
    combined   = concat([features[nodes], neigh_mean], 1) # [B, 2F]
    out        = relu(weight @ combined.T)                # [E, B]

Sharding: data-parallel over the node batch B=50000 across 8 cores (6250
nodes each, padded to 6272 = 49 tiles of 128); features table (bf16) and
weight replicated per core.

The kernel is bound by the SWDGE descriptor-generation fixed cost (~1us
per indirect DMA on the Pool engine): each indirect gather carries at
most one row index per partition (128 rows), so the 6250*17 = 106250 row
fetches per core need at least ceil(106250/128) = 831 gather
instructions — exactly what this kernel issues (48 full slot-major tiles
x 17 + 15 node-major runt blocks). Everything else (PE, ACT, DMA
transfer) is hidden under the serialized Pool-engine time; the remaining
overhead is first-gather startup latency and the last tile's dependency
tail, both minimized by interleaving and split stores below.

Per 128-node tile on each core (gathers bf16, PSUM fp32):
  1. 17 indirect-DMA gathers (one per row slot: self + 16 neighbors) pull
     512B feature rows into g [128, 17*256] (partition = node); each
     neighbor gather is chased by its transpose-accumulate matmuls so the
     PE trails the Pool engine by one slot.
  2. TensorE matmuls with an identity rhs transpose the self chunks and
     transpose-ACCUMULATE the neighbor chunks into one PSUM bank
     (combined^T; one accumulation group: single start/stop, lazy zero
     covers each region's first touch; the 1/16 mean factor is pre-folded
     into the neighbor half of the weight on the host).
  3. ACT copies the self half of combined^T to SBUF early (its finals run
     under the neighbor gathers); the neighbor half follows at tile end.
  4. TensorE multiplies with pre-swizzled W^T chunks accumulating over the
     four 128-feature chunks -> psum [128 nodes, 256 emb] fp32, split in
     two embed halves so relu/store of half 0 overlaps half 1.
  5. ACT relu-copies to SBUF bf16, DMA to out_t [6272, 256].

Host assembles: concat core outputs' first 6250 rows, transpose, cast to
fp32 -> [256, B].
"""

import numpy as np

P = 128      # nodes per tile / partitions
F = 256      # feature dim
S = 16       # sampled neighbors
E = 256      # embed dim
K = 1 + S    # gathered rows per node
V = 100000   # feature table rows
B_FULL = 50000
N_CORES = 8
B_CORE = B_FULL // N_CORES          # 6250
T = (B_CORE + P - 1) // P           # 49 tiles
B_PAD = T * P                       # 6272
TF = B_CORE // P                    # 48 full tiles
R_NODES = B_CORE - TF * P           # 106 nodes in the runt tile
R_TOK = R_NODES * K                 # 1802 runt tokens
NB = (R_TOK + P - 1) // P           # 15 node-major runt gather blocks
NCOL = TF * K + NB                  # 831 index columns

_prog_cache = {}


def _build_program(reps=1):
    import concourse.bass as bass
    import concourse.mybir as mybir
    import concourse.tile as tile
    from concourse import bacc

    FP = mybir.dt.float32
    BF = mybir.dt.bfloat16
    nc = bacc.Bacc("TRN2", num_devices=N_CORES)

    feat = nc.dram_tensor("feat", [V, F], BF, kind="ExternalInput")
    idx_r = nc.dram_tensor("idx_r", [P, NCOL], mybir.dt.int32,
                           kind="ExternalInput")
    wt_r = nc.dram_tensor("wt_r", [P, 4 * E], BF, kind="ExternalInput")
    # aux: identity [P, P] ++ per-runt-block selection matrices
    # [P, NB * 18] (9 self-router + 9 neighbor-sum columns per block)
    # ++ zeros [P, P] (lhsT of the runt PSUM-bank pre-zeroing matmul).
    aux_r = nc.dram_tensor("aux_r", [P, P + NB * 18 + P], BF,
                           kind="ExternalInput")
    out_t = nc.dram_tensor("out_t", [B_PAD, E], BF, kind="ExternalOutput")

    with tile.TileContext(nc) as tc:
        with tc.tile_pool(name="const", bufs=1) as const, \
             tc.tile_pool(name="gpool", bufs=6) as gpool, \
             tc.tile_pool(name="wpool", bufs=3) as wpool, \
             tc.tile_pool(name="ppool", bufs=2, space="PSUM") as ppool:
            idx_sb = const.tile([P, NCOL], mybir.dt.int32)
            # The runt tile runs FIRST (so the schedule's tail is a full
            # tile's shorter dependency chain): head-load its index columns,
            # then the constants it needs, then the bulk of the indices.
            nc.sync.dma_start(out=idx_sb[:, TF * K:],
                              in_=idx_r.ap()[:, TF * K:])
            rsel = const.tile([P, NB * 18], BF)
            nc.sync.dma_start(out=rsel[:], in_=aux_r.ap()[:, P:P + NB * 18])
            zeros = const.tile([P, P], BF)
            nc.sync.dma_start(out=zeros[:], in_=aux_r.ap()[:, P + NB * 18:])
            wt_sb = const.tile([P, 4 * E], BF)
            nc.sync.dma_start(out=wt_sb[:], in_=wt_r.ap())
            ident = const.tile([P, P], BF)
            nc.sync.dma_start(out=ident[:], in_=aux_r.ap()[:, 0:P])
            nc.sync.dma_start(out=idx_sb[:, :TF * K],
                              in_=idx_r.ap()[:, :TF * K])

            for rep in range(reps):
              # Runt tile (106 nodes, 1802 tokens): tokens are node-major
              # packed so only ceil(1802/128)=15 gathers are needed instead
              # of 17. Constant 0/1 selection matrices (rsel) route each
              # token's transposed features to its node column: self tokens
              # via a one-hot router, neighbor tokens summed by column. The
              # bank is pre-zeroed by one zeros-lhsT matmul so every
              # selection matmul is a pure accumulate (a node's 17 tokens
              # can straddle two gather blocks).
              g = gpool.tile([P, K * F], BF, tag="g")
              ct_ps = ppool.tile([P, 4 * P], FP, tag="ct")
              nc.tensor.matmul(
                  ct_ps[:], lhsT=zeros[:], rhs=wt_sb[:, 0:4 * P],
                  start=True, stop=False, skip_group_check=True)
              for q in range(NB):
                  nc.gpsimd.indirect_dma_start(
                      out=g[:, q * F:(q + 1) * F], out_offset=None,
                      in_=feat.ap(),
                      in_offset=bass.IndirectOffsetOnAxis(
                          ap=idx_sb[:, TF * K + q:TF * K + q + 1], axis=0))
                  base = (P * q) // K
                  w = min(P * q + P - 1, R_TOK - 1) // K - base + 1
                  for c in range(2):
                      lhs = g[:, q * F + c * P:q * F + (c + 1) * P]
                      nc.tensor.matmul(
                          ct_ps[:, c * P + base:c * P + base + w],
                          lhsT=lhs, rhs=rsel[:, q * 18:q * 18 + w],
                          start=False, stop=False, skip_group_check=True)
                      nc.tensor.matmul(
                          ct_ps[:, (2 + c) * P + base:(2 + c) * P + base + w],
                          lhsT=lhs, rhs=rsel[:, q * 18 + 9:q * 18 + 9 + w],
                          start=False,
                          stop=(q == NB - 1 and c == 1),
                          skip_group_check=True)
              ct = wpool.tile([P, 4 * P], BF, tag="ct_sb")
              nc.scalar.activation(ct[:], ct_ps[:],
                                   mybir.ActivationFunctionType.Copy)
              psum_o = ppool.tile([P, E], FP, tag="po")
              ot = wpool.tile([P, E], BF, tag="ot")
              for h in range(2):
                  for c in range(4):
                      nc.tensor.matmul(
                          psum_o[:, h * P:(h + 1) * P],
                          lhsT=ct[:, c * P:(c + 1) * P],
                          rhs=wt_sb[:, c * E + h * P:c * E + (h + 1) * P],
                          start=(h == 0 and c == 0),
                          stop=(h == 1 and c == 3),
                          skip_group_check=True)
                  nc.scalar.activation(ot[:, h * P:(h + 1) * P],
                                       psum_o[:, h * P:(h + 1) * P],
                                       mybir.ActivationFunctionType.Relu)
                  nc.sync.dma_start(
                      out=out_t.ap()[TF * P:T * P, h * P:(h + 1) * P],
                      in_=ot[:, h * P:(h + 1) * P])

              for t in range(TF):
                g = gpool.tile([P, K * F], BF, tag="g")
                ct_ps = ppool.tile([P, 4 * P], FP, tag="ct")
                nc.gpsimd.indirect_dma_start(
                    out=g[:, 0:F], out_offset=None,
                    in_=feat.ap(),
                    in_offset=bass.IndirectOffsetOnAxis(
                        ap=idx_sb[:, t * K:t * K + 1], axis=0))
                # One PSUM accumulation group covers the whole ct_ps bank:
                # the first matmul opens it (start), the last closes it
                # (stop); in between, the first touch of each byte range
                # after the start writes (lazy zero) and repeat touches
                # accumulate — giving self transposes (touched once) and
                # neighbor transpose-accumulates (touched 16x) in one pass.
                for c in range(2):
                    nc.tensor.matmul(
                        ct_ps[:, c * P:(c + 1) * P],
                        lhsT=g[:, c * P:(c + 1) * P], rhs=ident[:],
                        start=(c == 0), stop=False, skip_group_check=True)
                ct = wpool.tile([P, 4 * P], BF, tag="ct_sb")
                # Self half of combined^T is complete already — copy it out
                # and start the finals' self-chunk matmuls while neighbor
                # gathers still stream; only the neighbor half of the copy
                # and the last two final chunks remain on the tail.
                nc.scalar.activation(ct[:, 0:2 * P], ct_ps[:, 0:2 * P],
                                     mybir.ActivationFunctionType.Copy)
                psum_o = ppool.tile([P, E], FP, tag="po")
                ot = wpool.tile([P, E], BF, tag="ot")
                # Interleave neighbor gathers with transpose-accumulate
                # matmuls so the PE trails the Pool engine by one gather and
                # the end-of-program tail is short.
                for k in range(S):
                    j = 1 + k
                    nc.gpsimd.indirect_dma_start(
                        out=g[:, j * F:(j + 1) * F], out_offset=None,
                        in_=feat.ap(),
                        in_offset=bass.IndirectOffsetOnAxis(
                            ap=idx_sb[:, t * K + j:t * K + j + 1], axis=0))
                    for c in range(2):
                        nc.tensor.matmul(
                            ct_ps[:, (2 + c) * P:(3 + c) * P],
                            lhsT=g[:, j * F + c * P:j * F + (c + 1) * P],
                            rhs=ident[:],
                            start=False, stop=(k == S - 1 and c == 1),
                            skip_group_check=True)
                    if k < 2:
                        # Final-matmul self chunk k for both embed halves
                        # (single psum_o accumulation group: one start, one
                        # stop, lazy-zero handles each half's first touch).
                        for h in range(2):
                            nc.tensor.matmul(
                                psum_o[:, h * P:(h + 1) * P],
                                lhsT=ct[:, k * P:(k + 1) * P],
                                rhs=wt_sb[:, k * E + h * P:k * E + (h + 1) * P],
                                start=(k == 0 and h == 0), stop=False,
                                skip_group_check=True)
                nc.scalar.activation(ct[:, 2 * P:4 * P], ct_ps[:, 2 * P:4 * P],
                                     mybir.ActivationFunctionType.Copy)
                # Remaining final chunks (neighbor half), then per-half
                # relu/store so the first store overlaps the second half.
                for h in range(2):
                    for c in range(2, 4):
                        nc.tensor.matmul(
                            psum_o[:, h * P:(h + 1) * P],
                            lhsT=ct[:, c * P:(c + 1) * P],
                            rhs=wt_sb[:, c * E + h * P:c * E + (h + 1) * P],
                            start=False, stop=(h == 1 and c == 3),
                            skip_group_check=True)
                    nc.scalar.activation(ot[:, h * P:(h + 1) * P],
                                         psum_o[:, h * P:(h + 1) * P],
                                         mybir.ActivationFunctionType.Relu)
                    nc.sync.dma_start(
                        out=out_t.ap()[t * P:(t + 1) * P, h * P:(h + 1) * P],
                        in_=ot[:, h * P:(h + 1) * P])

    nc.compile()
    return nc


def get_program(reps=1):
    key = ("nc", reps)
    if key not in _prog_cache:
        _prog_cache[key] = _build_program(reps)
    return _prog_cache[key]


def _bf16(a):
    import ml_dtypes
    return np.asarray(a, dtype=np.float32).astype(ml_dtypes.bfloat16)


def _prep_idx(nodes_c, neigh_c):
    """Per-core [b] + [b, S] indices -> [P, NCOL] int32: 48 slot-major
    full tiles followed by NB node-major-packed runt gather columns."""
    idx_all = np.concatenate([nodes_c[:, None], neigh_c], axis=1)  # [b, K]
    full = (idx_all[:TF * P]
            .reshape(TF, P, K).transpose(1, 0, 2).reshape(P, TF * K))
    # Runt: token i (node-major) -> partition i % P, column i // P.
    toks = np.zeros(NB * P, np.int32)
    toks[:R_TOK] = idx_all[TF * P:].reshape(-1)
    runt = toks.reshape(NB, P).T
    return np.ascontiguousarray(
        np.concatenate([full, runt], axis=1).astype(np.int32))


def _prep_aux():
    """identity ++ runt selection matrices ++ zeros, [P, P+NB*18+P] bf16."""
    rsel = np.zeros((P, NB * 18), np.float32)
    for q in range(NB):
        base = (P * q) // K
        for p in range(P):
            i = P * q + p
            if i >= R_TOK:
                continue
            n, s = divmod(i, K)
            rsel[p, q * 18 + (0 if s == 0 else 9) + (n - base)] = 1.0
    return np.ascontiguousarray(_bf16(np.concatenate(
        [np.eye(P, dtype=np.float32), rsel, np.zeros((P, P), np.float32)],
        axis=1)))


def _prep_weight(weight):
    """[E, 2F] -> chunk-swizzled W.T [P, 4*E] bf16 with mean pre-folded."""
    wt = np.asarray(weight, dtype=np.float32).T.copy()   # [2F, E]
    wt[F:] /= S
    return np.ascontiguousarray(
        _bf16(wt.reshape(4, P, E).transpose(1, 0, 2).reshape(P, 4 * E)))


def make_in_maps(nodes, neigh_idx, features, weight):
    nodes = np.asarray(nodes)
    neigh_idx = np.asarray(neigh_idx)
    features = np.ascontiguousarray(_bf16(features))
    wt_r = _prep_weight(weight)
    aux_r = _prep_aux()
    in_maps = []
    for c in range(N_CORES):
        sl = slice(c * B_CORE, (c + 1) * B_CORE)
        idx_r = _prep_idx(nodes[sl].astype(np.int32),
                          neigh_idx[sl].astype(np.int32))
        in_maps.append({"feat": features, "idx_r": idx_r, "wt_r": wt_r,
                        "aux_r": aux_r})
    return in_maps


def kernel(nodes, neigh_idx, features, weight):
    import concourse.bass_utils as bass_utils

    assert np.asarray(nodes).shape[0] == B_FULL, "kernel hardcodes B=50000"
    nc = get_program()
    in_maps = make_in_maps(nodes, neigh_idx, features, weight)
    res = bass_utils.run_bass_kernel_spmd(
        nc, in_maps, core_ids=list(range(N_CORES)))
    out_t = np.concatenate(
        [np.asarray(res.results[c]["out_t"][:B_CORE], dtype=np.float32)
         for c in range(N_CORES)], axis=0)
    return np.ascontiguousarray(out_t.T)
